# revision 42
# baseline (speedup 1.0000x reference)
"""Trainium2 Bass kernel for nn_AttentionHead (B=8, N=2048, D=512, d=64).

Reference semantics (faithful to the torch original):
    K = key_input   @ W_key        # note: W_key used for Q, K AND V
    Q = query_input @ W_key
    V = value_input @ W_key
    S = Q @ K^T / sqrt(512)        # scaled by INPUT dim, not head dim
    S = mask(padding), causal-mask if masked_attention
    out = softmax(S) @ V

Sharding: pure data parallelism over batch — core b computes batch element b.
No collectives. Host-side prep is layout only (transpose + fp8 cast); every
FLOP of the math runs on-device.

Device algorithm (per core):
  - Q/K inputs arrive fp8e4m3, V in bf16 (V-entry quantization passes ~1:1
    to the output; Q/K only perturb scores — combined ~0.7% l2), host-packed
    [p, quarter, c, q] so each DMA line is 2-4 KiB contiguous; one dma_start
    per (input, quarter), issue spread over sync/scalar/gpsimd from t=0
  - Q/K projections run fp8 DoubleRow (virtual 256-deep contraction, 2
    matmuls per 512-wide n-slice); V projects bf16 4-chunk; QT/KT
    [64->dup 128, 2048] with W host-duplicated [W|W]
  - V natural [128, 65] tiles via PE-transpose (bf16) with a ones column
    (softmax denominators come free as row 64 of the PV matmul)
  - streaming order: projection slice ns, then attention q-block qb=ns —
    attention for qb only needs data slices <= qb, so the PE instruction
    stream never blocks on DMA for later data; the engine stays dense,
    which also keeps the HAM clock gate at 2.4 GHz longer (each idle
    window it sees drops the PE to 1.2 GHz for 3.4+ us)
  - per k-chunk j: S.T tile [k=128, q<=512] = KT_j.T @ QT_qb (exact-causal
    widths); pair (2t, 2t+1) S matmuls adjacent so they row-pack into
    disjoint PE row groups; exp via ACT straight PSUM->SBUF bf16 with
    1/sqrt(512) folded in; diagonal blocks masked by DVE tri-multiply
  - O.T [65, q] += V'_j.T @ P.T accumulated in PSUM over the q-block's
    k-chunks, epilogue (PE-transpose, divide by sums row, bf16 DMA out on
    gpsimd) immediately after — o-psum recycles every q-block
"""

import math

import numpy as np
import ml_dtypes

import concourse.bass as bass
import concourse.tile as tile
from concourse import bacc, mybir
from concourse import masks
from concourse.bass_utils import run_bass_kernel_spmd

P = 128            # partitions / k-chunk size
N = 2048           # sequence length
D = 512            # embedding dim
DH = 64            # head dim
EC = D // P        # 4 e-chunks for the projection contraction
KC = N // P        # 16 k-chunks
QW = 512           # q block width
NQB = N // QW      # 4 q blocks / n slices
SCALE = 1.0 / math.sqrt(float(D))

BF16 = mybir.dt.bfloat16
F32 = mybir.dt.float32
FP8 = mybir.dt.float8e4
DR = mybir.MatmulPerfMode.DoubleRow

_BUILD_CACHE = {}

OPTS = {
    "sbufs": 4,            # s psum pool buffers
    "jbufs": 2,            # proj/transpose psum pool buffers
    "obufs": 2,            # o accumulator buffers (one live + one in epilogue)
}


def _ensure_ntff_hook():
    """Install the antenv.axon_hooks shim so trace=True works under axon."""
    try:
        import antenv.axon_hooks  # noqa: F401
        return
    except ImportError:
        pass
    import sys
    import types

    try:
        from trn_agent_boot.trn_boot import _ntff_profile_via_ctypes
        hook = _ntff_profile_via_ctypes("/opt/axon/libaxon_pjrt.so")
    except Exception:
        hook = None
    mod = types.ModuleType("antenv.axon_hooks")
    state = {"hook": hook}
    mod.get_axon_ntff_profile_hook = lambda: state["hook"]
    mod.set_axon_ntff_profile_hook = lambda h: state.update(hook=h)
    sys.modules["antenv.axon_hooks"] = mod
    import antenv

    antenv.axon_hooks = mod


def _build(causal: bool, has_padding: bool):
    nc = bacc.Bacc("TRN2", target_bir_lowering=False, debug=False, num_devices=8)

    # host layout [p, quarter, c, q]: line per (p, quarter) is [c, q],
    # 2 KiB fp8 for Q/K, 4 KiB bf16 for V (V-entry quantization passes ~1:1
    # to the output, fp8 there would blow the error budget).
    xq_d = nc.dram_tensor("xq_t", [P, NQB * EC * QW], FP8, kind="ExternalInput")
    xk_d = nc.dram_tensor("xk_t", [P, NQB * EC * QW], FP8, kind="ExternalInput")
    xv_d = nc.dram_tensor("xv_t", [P, NQB * EC * QW], BF16, kind="ExternalInput")
    # w host-packed [p, c, 2*DH] = [W | W] so the Q/K projections emit
    # [128, q] tiles whose partition halves are copies — S matmul pairs can
    # then row-pack into disjoint PE row groups with no cross-partition copy.
    w_d = nc.dram_tensor("w", [P, EC * 2 * DH], FP8, kind="ExternalInput")
    wv_d = nc.dram_tensor("wv", [P, EC * DH], BF16, kind="ExternalInput")
    if has_padding:
        km_d = nc.dram_tensor("kmask", [KC, P], F32, kind="ExternalInput")
    # bf16 output: quantization adds ~0.2% (quadrature, well inside the
    # 2e-2 gate) and halves the output DMA traffic + descriptor cost
    out_d = nc.dram_tensor("out", [N, DH], BF16, kind="ExternalOutput")

    with tile.TileContext(nc) as tc:
        with (
            tc.tile_pool(name="const", bufs=1) as cpool,
            tc.tile_pool(name="x", bufs=12) as xpool,
            tc.tile_pool(name="big", bufs=1) as bigpool,
            tc.tile_pool(name="p", bufs=8) as ppool,
            tc.tile_pool(name="epi", bufs=2) as epipool,
            tc.tile_pool(name="o", bufs=OPTS["obufs"], space="PSUM") as opool,
            tc.tile_pool(name="s", bufs=OPTS["sbufs"], space="PSUM") as spool,
            tc.tile_pool(name="j", bufs=OPTS["jbufs"], space="PSUM") as jpool,
        ):
            # --- weight + input DMAs issued before anything else so the
            # queues start moving within ~1us of kernel entry. w/wv on
            # gpsimd (tiny, needed by projection 0); q quarters lead on
            # sync so the projection pipeline unblocks earliest. Issue is
            # spread over three engines so descriptor generation
            # (~0.6-1us per call) runs concurrently. 2-4 KiB lines.
            # No PE warmup junk: measured on HW it neither holds the HAM
            # clock gate warm through the DMA window nor speeds the real
            # work — it only adds PE time. ---
            w_sb = cpool.tile([P, EC, 2 * DH], FP8)
            nc.gpsimd.dma_start(
                w_sb[:], w_d.ap().rearrange("p (c d) -> p c d", c=EC)
            )
            wv_sb = cpool.tile([P, EC, DH], BF16)
            nc.gpsimd.dma_start(
                wv_sb[:], wv_d.ap().rearrange("p (c d) -> p c d", c=EC)
            )
            if has_padding:
                km_sb = cpool.tile([P, KC], F32)
                nc.gpsimd.dma_start(km_sb[:], km_d.ap().transpose([1, 0]))

            engs = {"q": nc.sync, "k": nc.scalar, "v": nc.gpsimd}
            dts = {"q": FP8, "k": FP8, "v": BF16}
            x_sb = {}
            for nh in range(NQB):
                for tname, xd in (("q", xq_d), ("k", xk_d), ("v", xv_d)):
                    t = xpool.tile([P, EC, QW], dts[tname], tag=f"x{tname}")
                    engs[tname].dma_start(
                        t[:],
                        xd.ap()[:, nh * EC * QW:(nh + 1) * EC * QW].rearrange(
                            "p (c q) -> p c q", c=EC
                        ),
                    )
                    x_sb[(tname, nh)] = t

            # --- ACT warmup (hide exp table load behind the DMA window) ---
            warm = cpool.tile([P, 1], F32)
            nc.vector.memset(warm[:], 0.0)
            nc.scalar.activation(warm[:], warm[:], mybir.ActivationFunctionType.Exp)

            ident = cpool.tile([P, P], F32)
            masks.make_identity(nc, ident[:])
            identb = cpool.tile([P, P], BF16)
            masks.make_identity(nc, identb[:])
            # upper-triangular (incl diag) 0/1 mask in [k, q] coords for the
            # causal diagonal blocks; multiply on DVE (gpsimd's slow semaphore
            # handling would sit in the exp->PV chain otherwise)
            tri = cpool.tile([P, P], BF16)
            masks.make_upper_triangular(nc, tri[:], val=1.0, diag=True)

            qt = bigpool.tile([P, N], BF16, tag="qt")   # rows 0-63 QT, 64-127 dup
            kt = bigpool.tile([P, N], BF16, tag="kt")
            vt = bigpool.tile([DH, N], BF16, tag="vt")
            v_sb = bigpool.tile([P, KC, DH + 1], BF16, tag="vn")
            # ones column (softmax denominator rider) set once, in SBUF —
            # a strided bf16 memset into PSUM fails the ISA 4B-cell check
            nc.vector.memset(v_sb[:, :, DH], 1.0)

            # --- streaming structure: projection slice ns, then attention
            # q-block qb == ns. Attention for qb only needs projected data
            # from slices <= qb, so the PE instruction stream never waits on
            # DMA for data later in the stream — the engine stays dense from
            # the first quarter's arrival, HAM warms early, and the whole
            # attention phase runs inside the warm-clock window. Each
            # o-accumulator is freed by its epilogue before the next q-block
            # needs one (obufs=2), which frees PSUM for a 4-deep S pipeline.

            def emit_proj(ns):
                sl = slice(ns * QW, (ns + 1) * QW)
                for tname in ("q", "k", "v"):
                    ps = jpool.tile([P, QW], F32, tag="j")
                    if tname == "v":
                        # bf16 4-chunk contraction (fp8 V would pass its
                        # quantization error 1:1 to the output)
                        for c in range(EC):
                            nc.tensor.matmul(
                                ps[:DH, :],
                                wv_sb[:, c, :],
                                x_sb[(tname, ns)][:, c, :],
                                start=(c == 0),
                                stop=(c == EC - 1),
                            )
                    else:
                        # fp8 DoubleRow: virtual 256-deep contraction
                        for cc in range(EC // 2):
                            nc.tensor.matmul(
                                ps[:P, :],
                                w_sb[:, 2 * cc:2 * cc + 2, :],
                                x_sb[(tname, ns)][:, 2 * cc:2 * cc + 2, :],
                                start=(cc == 0),
                                stop=(cc == EC // 2 - 1),
                                perf_mode=DR,
                            )
                    if tname == "q":
                        nc.vector.tensor_copy(qt[:, sl], ps[:])
                    elif tname == "k":
                        # keep ACT free for the exp stream — with the
                        # interleaved slice/q-block order ACT is co-critical
                        nc.vector.tensor_copy(kt[:, sl], ps[:])
                    else:
                        nc.vector.tensor_copy(vt[:, sl], ps[:DH, :])
                # V natural tiles for this n-slice: PE transpose + ones column
                vtp = jpool.tile([P, NQB, DH + 2], BF16, tag="j")
                for i in range(NQB):
                    j = ns * NQB + i
                    nc.tensor.transpose(
                        vtp[:, i, :DH], vt[:, j * P:(j + 1) * P], identb[:DH, :DH]
                    )
                nc.vector.tensor_copy(
                    v_sb[:, ns * NQB:(ns + 1) * NQB, :DH], vtp[:, :, :DH]
                )

            def emit_s(j, qb, idx, p_tiles):
                base = DH * idx
                q_off = max(0, j * P - qb * QW) if causal else 0
                width = QW - q_off
                s_ps = spool.tile([P, QW], F32, tag="s", name=f"s{j}_{qb}")
                nc.tensor.matmul(
                    s_ps[:, :width],
                    kt[base:base + DH, j * P:(j + 1) * P],
                    qt[base:base + DH, qb * QW + q_off:(qb + 1) * QW],
                    start=True,
                    stop=True,
                )
                p_sb = ppool.tile([P, QW], BF16, tag="p", name=f"p{j}_{qb}")
                nc.scalar.activation(
                    p_sb[:, :width],
                    s_ps[:, :width],
                    mybir.ActivationFunctionType.Exp,
                    scale=SCALE,
                )
                if causal and qb == j // NQB:
                    # diagonal block at cols [0,128): keep q_loc >= k_loc
                    nc.vector.tensor_mul(p_sb[:, :P], p_sb[:, :P], tri[:])
                if has_padding:
                    nc.vector.tensor_scalar_mul(
                        p_sb[:, :width], p_sb[:, :width], km_sb[:, j:j + 1]
                    )
                p_tiles[(j, qb)] = (p_sb, q_off, width)

            for ns in range(NQB):
                emit_proj(ns)

                qb = ns
                j_last = (QW // P) * (qb + 1) - 1 if causal else KC - 1
                o_t = opool.tile([DH + 1, QW], F32, tag="o", name=f"o{qb}")
                p_tiles = {}
                for t in range((j_last + 1) // 2):
                    js = (2 * t, 2 * t + 1)
                    # the pair's S matmuls are adjacent so they row-pack
                    # (rows 0-63 / 64-127 run concurrent); PVs chain after
                    for idx, j in enumerate(js):
                        emit_s(j, qb, idx, p_tiles)
                    for j in js:
                        p_sb, q_off, width = p_tiles.pop((j, qb))
                        nc.tensor.matmul(
                            o_t[:, q_off:QW],
                            v_sb[:, j, :],
                            p_sb[:, :width],
                            start=(j == 0),
                            stop=(j == j_last),
                        )

                # epilogue for this q-block, pipelined per 128-row chunk
                # (copy -> PE-transpose -> recip -> scale -> DMA) so the
                # final q-block's output overlaps its own processing instead
                # of one serial ~3us chain at the very end of the kernel
                oT = epipool.tile([DH + 1, QW], F32, tag="ot")
                for i in range(NQB):
                    nc.vector.tensor_copy(
                        oT[:, i * P:(i + 1) * P], o_t[:, i * P:(i + 1) * P]
                    )
                    etp = jpool.tile([P, DH + 1], F32, tag="j",
                                     name=f"etp{qb}_{i}")
                    nc.tensor.transpose(
                        etp[:], oT[:, i * P:(i + 1) * P],
                        ident[:DH + 1, :DH + 1],
                    )
                    recip = epipool.tile([P, 1], F32, tag="recip",
                                         name=f"rec{qb}_{i}")
                    nc.vector.reciprocal(recip[:], etp[:, DH:DH + 1])
                    o_sb = epipool.tile([P, DH], BF16, tag="osb",
                                        name=f"osb{qb}_{i}")
                    nc.vector.tensor_scalar_mul(o_sb[:], etp[:, :DH], recip[:])
                    row = (qb * NQB + i) * P
                    nc.gpsimd.dma_start(out_d.ap()[row:row + P, :], o_sb[:])

    nc.compile()
    return nc


def _get(causal: bool, has_padding: bool):
    key = (causal, has_padding)
    if key not in _BUILD_CACHE:
        _BUILD_CACHE[key] = _build(causal, has_padding)
    return _BUILD_CACHE[key]


def _pack_x(x, dtype):
    # x [N, D] f32 -> x.T [D, N] with d = c*P + p, n = nh*QW + q
    # -> [p, nh, c, q], so each (p, nh) line is [c, q] contiguous (2-4 KiB)
    xt = np.asarray(x, dtype=np.float32).T.astype(dtype)        # [D, N]
    xt = xt.reshape(EC, P, NQB, QW).transpose(1, 2, 0, 3)       # [p, nh, c, q]
    return np.ascontiguousarray(xt).reshape(P, NQB * EC * QW)


def run(key_input, query_input, value_input, padding_mask, masked_attention,
        W_key, W_query=None, W_value=None, trace=False, **_ignored):
    key_input = np.asarray(key_input, dtype=np.float32)
    query_input = np.asarray(query_input, dtype=np.float32)
    value_input = np.asarray(value_input, dtype=np.float32)
    padding_mask = np.asarray(padding_mask)
    W_key = np.asarray(W_key, dtype=np.float32)

    B = key_input.shape[0]
    causal = bool(int(np.asarray(masked_attention)))
    has_padding = bool(padding_mask.any())
    nc = _get(causal, has_padding)

    f8 = ml_dtypes.float8_e4m3
    bf = ml_dtypes.bfloat16
    wcat = np.concatenate([W_key, W_key], axis=1).astype(f8)    # [D, 2*DH]
    w_b = np.ascontiguousarray(
        wcat.reshape(EC, P, 2 * DH).transpose(1, 0, 2)
    ).reshape(P, EC * 2 * DH)
    wv_b = np.ascontiguousarray(
        W_key.astype(bf).reshape(EC, P, DH).transpose(1, 0, 2)
    ).reshape(P, EC * DH)
    in_maps = []
    for b in range(B):
        m = {
            "xq_t": _pack_x(query_input[b], f8),
            "xk_t": _pack_x(key_input[b], f8),
            "xv_t": _pack_x(value_input[b], bf),
            "w": w_b,
            "wv": wv_b,
        }
        if has_padding:
            # multiplicative key mask in [KC, P] layout: 0 where padded
            km = (~padding_mask[b].reshape(N)).astype(np.float32)
            m["kmask"] = np.ascontiguousarray(km.reshape(KC, P))
        in_maps.append(m)

    if trace:
        _ensure_ntff_hook()
    res = run_bass_kernel_spmd(nc, in_maps, core_ids=list(range(B)), trace=trace)
    out = np.stack([np.asarray(res.results[b]["out"]) for b in range(B)], axis=0)
    return out.astype(np.float32), res


def kernel(**inputs) -> np.ndarray:
    out, _ = run(**inputs)
    return out


# revision 43
# speedup vs baseline: 1.2170x; 1.2170x over previous
"""Trainium2 Bass kernel for nn_AttentionHead (B=8, N=2048, D=512, d=64).

Reference semantics (faithful to the torch original):
    K = key_input   @ W_key        # note: W_key used for Q, K AND V
    Q = query_input @ W_key
    V = value_input @ W_key
    S = Q @ K^T / sqrt(512)        # scaled by INPUT dim, not head dim
    S = mask(padding), causal-mask if masked_attention
    out = softmax(S) @ V

Sharding: pure data parallelism over batch — core b computes batch element b.
No collectives. Host-side prep is layout only (transpose + fp8 cast); every
FLOP of the math runs on-device.

Device algorithm (per core):
  - Q/K inputs arrive fp8e4m3, V in bf16 (V-entry quantization passes ~1:1
    to the output; Q/K only perturb scores — combined ~0.7% l2), host-packed
    [p, quarter, c, q] so each DMA line is 2-4 KiB contiguous; one dma_start
    per (input, quarter), issue spread over sync/scalar/gpsimd from t=0
  - Q/K projections run fp8 DoubleRow (virtual 256-deep contraction, 2
    matmuls per 512-wide n-slice); V projects bf16 4-chunk; QT/KT
    [64->dup 128, 2048] with W host-duplicated [W|W]
  - V natural [128, 65] tiles via PE-transpose (bf16) with a ones column
    (softmax denominators come free as row 64 of the PV matmul)
  - streaming order: projection slice ns, then attention q-block qb=ns —
    attention for qb only needs data slices <= qb, so the PE instruction
    stream never blocks on DMA for later data; the engine stays dense,
    which also keeps the HAM clock gate at 2.4 GHz longer (each idle
    window it sees drops the PE to 1.2 GHz for 3.4+ us)
  - per k-chunk j: S.T tile [k=128, q<=512] = KT_j.T @ QT_qb (exact-causal
    widths); pair (2t, 2t+1) S matmuls adjacent so they row-pack into
    disjoint PE row groups; exp via ACT straight PSUM->SBUF bf16 with
    1/sqrt(512) folded in; diagonal blocks masked by DVE tri-multiply
  - O.T [65, q] += V'_j.T @ P.T accumulated in PSUM over the q-block's
    k-chunks, epilogue (PE-transpose, divide by sums row, bf16 DMA out on
    gpsimd) immediately after — o-psum recycles every q-block
"""

import math

import numpy as np
import ml_dtypes

import concourse.bass as bass
import concourse.tile as tile
from concourse import bacc, mybir
from concourse import masks
from concourse.bass_utils import run_bass_kernel_spmd

P = 128            # partitions / k-chunk size
N = 2048           # sequence length
D = 512            # embedding dim
DH = 64            # head dim
EC = D // P        # 4 e-chunks for the projection contraction
KC = N // P        # 16 k-chunks
QW = 512           # q block width
NQB = N // QW      # 4 q blocks / n slices
SCALE = 1.0 / math.sqrt(float(D))

BF16 = mybir.dt.bfloat16
F32 = mybir.dt.float32
FP8 = mybir.dt.float8e4
DR = mybir.MatmulPerfMode.DoubleRow

_BUILD_CACHE = {}

OPTS = {
    "sbufs": 4,            # s psum pool buffers
    "jbufs": 2,            # proj/transpose psum pool buffers
    "obufs": 2,            # o accumulator buffers (one live + one in epilogue)
}


def _ensure_ntff_hook():
    """Install the antenv.axon_hooks shim so trace=True works under axon."""
    try:
        import antenv.axon_hooks  # noqa: F401
        return
    except ImportError:
        pass
    import sys
    import types

    try:
        from trn_agent_boot.trn_boot import _ntff_profile_via_ctypes
        hook = _ntff_profile_via_ctypes("/opt/axon/libaxon_pjrt.so")
    except Exception:
        hook = None
    mod = types.ModuleType("antenv.axon_hooks")
    state = {"hook": hook}
    mod.get_axon_ntff_profile_hook = lambda: state["hook"]
    mod.set_axon_ntff_profile_hook = lambda h: state.update(hook=h)
    sys.modules["antenv.axon_hooks"] = mod
    import antenv

    antenv.axon_hooks = mod


def _build(causal: bool, has_padding: bool):
    nc = bacc.Bacc("TRN2", target_bir_lowering=False, debug=False, num_devices=8)

    # host layout [p, quarter, c, q]: line per (p, quarter) is [c, q],
    # 2 KiB fp8 for Q/K, 4 KiB bf16 for V (V-entry quantization passes ~1:1
    # to the output, fp8 there would blow the error budget).
    xq_d = nc.dram_tensor("xq_t", [P, NQB * EC * QW], FP8, kind="ExternalInput")
    xk_d = nc.dram_tensor("xk_t", [P, NQB * EC * QW], FP8, kind="ExternalInput")
    xv_d = nc.dram_tensor("xv_t", [P, NQB * EC * QW], BF16, kind="ExternalInput")
    # w host-packed [p, c, 2*DH] = [W | W] so the Q/K projections emit
    # [128, q] tiles whose partition halves are copies — S matmul pairs can
    # then row-pack into disjoint PE row groups with no cross-partition copy.
    w_d = nc.dram_tensor("w", [P, EC * 2 * DH], FP8, kind="ExternalInput")
    wv_d = nc.dram_tensor("wv", [P, EC * DH], BF16, kind="ExternalInput")
    if has_padding:
        km_d = nc.dram_tensor("kmask", [KC, P], F32, kind="ExternalInput")
    # bf16 output: quantization adds ~0.2% (quadrature, well inside the
    # 2e-2 gate) and halves the output DMA traffic + descriptor cost
    out_d = nc.dram_tensor("out", [N, DH], BF16, kind="ExternalOutput")

    with tile.TileContext(nc) as tc:
        with (
            tc.tile_pool(name="const", bufs=1) as cpool,
            tc.tile_pool(name="x", bufs=12) as xpool,
            tc.tile_pool(name="big", bufs=1) as bigpool,
            tc.tile_pool(name="p", bufs=8) as ppool,
            tc.tile_pool(name="epi", bufs=2) as epipool,
            tc.tile_pool(name="o", bufs=OPTS["obufs"], space="PSUM") as opool,
            tc.tile_pool(name="s", bufs=OPTS["sbufs"], space="PSUM") as spool,
            tc.tile_pool(name="j", bufs=OPTS["jbufs"], space="PSUM") as jpool,
        ):
            # --- weight + input DMAs issued before anything else so the
            # queues start moving within ~1us of kernel entry. w/wv on
            # gpsimd (tiny, needed by projection 0); q quarters lead on
            # sync so the projection pipeline unblocks earliest. Issue is
            # spread over three engines so descriptor generation
            # (~0.6-1us per call) runs concurrently. 2-4 KiB lines.
            # No PE warmup junk: measured on HW it neither holds the HAM
            # clock gate warm through the DMA window nor speeds the real
            # work — it only adds PE time. ---
            w_sb = cpool.tile([P, EC, 2 * DH], FP8)
            nc.gpsimd.dma_start(
                w_sb[:], w_d.ap().rearrange("p (c d) -> p c d", c=EC)
            )
            wv_sb = cpool.tile([P, EC, DH], BF16)
            nc.gpsimd.dma_start(
                wv_sb[:], wv_d.ap().rearrange("p (c d) -> p c d", c=EC)
            )
            if has_padding:
                km_sb = cpool.tile([P, KC], F32)
                nc.gpsimd.dma_start(km_sb[:], km_d.ap().transpose([1, 0]))

            engs = {"q": nc.sync, "k": nc.scalar, "v": nc.gpsimd}
            dts = {"q": FP8, "k": FP8, "v": BF16}
            x_sb = {}
            for nh in range(NQB):
                for tname, xd in (("q", xq_d), ("k", xk_d), ("v", xv_d)):
                    t = xpool.tile([P, EC, QW], dts[tname], tag=f"x{tname}")
                    engs[tname].dma_start(
                        t[:],
                        xd.ap()[:, nh * EC * QW:(nh + 1) * EC * QW].rearrange(
                            "p (c q) -> p c q", c=EC
                        ),
                    )
                    x_sb[(tname, nh)] = t

            # --- ACT warmup (hide exp table load behind the DMA window) ---
            warm = cpool.tile([P, 1], F32)
            nc.vector.memset(warm[:], 0.0)
            nc.scalar.activation(warm[:], warm[:], mybir.ActivationFunctionType.Exp)

            ident = cpool.tile([P, P], F32)
            masks.make_identity(nc, ident[:])
            identb = cpool.tile([P, P], BF16)
            masks.make_identity(nc, identb[:])
            # upper-triangular (incl diag) 0/1 mask in [k, q] coords for the
            # causal diagonal blocks; multiply on DVE (gpsimd's slow semaphore
            # handling would sit in the exp->PV chain otherwise)
            tri = cpool.tile([P, P], BF16)
            masks.make_upper_triangular(nc, tri[:], val=1.0, diag=True)

            qt = bigpool.tile([P, N], BF16, tag="qt")   # rows 0-63 QT, 64-127 dup
            kt = bigpool.tile([P, N], BF16, tag="kt")
            vt = bigpool.tile([DH, N], BF16, tag="vt")
            v_sb = bigpool.tile([P, KC, DH + 1], BF16, tag="vn")
            # ones column (softmax denominator rider) set once, in SBUF —
            # a strided bf16 memset into PSUM fails the ISA 4B-cell check
            nc.vector.memset(v_sb[:, :, DH], 1.0)

            # --- streaming structure: projection slice ns, then attention
            # q-block qb == ns. Attention for qb only needs projected data
            # from slices <= qb, so the PE instruction stream never waits on
            # DMA for data later in the stream — the engine stays dense from
            # the first quarter's arrival, HAM warms early, and the whole
            # attention phase runs inside the warm-clock window. Each
            # o-accumulator is freed by its epilogue before the next q-block
            # needs one (obufs=2), which frees PSUM for a 4-deep S pipeline.

            def emit_proj(ns):
                sl = slice(ns * QW, (ns + 1) * QW)
                for tname in ("q", "k", "v"):
                    ps = jpool.tile([P, QW], F32, tag="j")
                    if tname == "v":
                        # bf16 4-chunk contraction (fp8 V would pass its
                        # quantization error 1:1 to the output)
                        for c in range(EC):
                            nc.tensor.matmul(
                                ps[:DH, :],
                                wv_sb[:, c, :],
                                x_sb[(tname, ns)][:, c, :],
                                start=(c == 0),
                                stop=(c == EC - 1),
                            )
                    else:
                        # fp8 DoubleRow: virtual 256-deep contraction
                        for cc in range(EC // 2):
                            nc.tensor.matmul(
                                ps[:P, :],
                                w_sb[:, 2 * cc:2 * cc + 2, :],
                                x_sb[(tname, ns)][:, 2 * cc:2 * cc + 2, :],
                                start=(cc == 0),
                                stop=(cc == EC // 2 - 1),
                                perf_mode=DR,
                            )
                    if tname == "q":
                        nc.vector.tensor_copy(qt[:, sl], ps[:])
                    elif tname == "k":
                        # keep ACT free for the exp stream — with the
                        # interleaved slice/q-block order ACT is co-critical
                        nc.vector.tensor_copy(kt[:, sl], ps[:])
                    else:
                        nc.vector.tensor_copy(vt[:, sl], ps[:DH, :])
                # V natural tiles for this n-slice: PE transpose + ones column
                vtp = jpool.tile([P, NQB, DH + 2], BF16, tag="j")
                for i in range(NQB):
                    j = ns * NQB + i
                    nc.tensor.transpose(
                        vtp[:, i, :DH], vt[:, j * P:(j + 1) * P], identb[:DH, :DH]
                    )
                nc.vector.tensor_copy(
                    v_sb[:, ns * NQB:(ns + 1) * NQB, :DH], vtp[:, :, :DH]
                )

            def emit_s(j, qb, idx, p_tiles):
                base = DH * idx
                q_off = max(0, j * P - qb * QW) if causal else 0
                width = QW - q_off
                s_ps = spool.tile([P, QW], F32, tag="s", name=f"s{j}_{qb}")
                nc.tensor.matmul(
                    s_ps[:, :width],
                    kt[base:base + DH, j * P:(j + 1) * P],
                    qt[base:base + DH, qb * QW + q_off:(qb + 1) * QW],
                    start=True,
                    stop=True,
                )
                p_sb = ppool.tile([P, QW], BF16, tag="p", name=f"p{j}_{qb}")
                nc.scalar.activation(
                    p_sb[:, :width],
                    s_ps[:, :width],
                    mybir.ActivationFunctionType.Exp,
                    scale=SCALE,
                )
                if causal and qb == j // NQB:
                    # diagonal block at cols [0,128): keep q_loc >= k_loc
                    nc.vector.tensor_mul(p_sb[:, :P], p_sb[:, :P], tri[:])
                if has_padding:
                    nc.vector.tensor_scalar_mul(
                        p_sb[:, :width], p_sb[:, :width], km_sb[:, j:j + 1]
                    )
                p_tiles[(j, qb)] = (p_sb, q_off, width)

            for ns in range(NQB):
                emit_proj(ns)

                qb = ns
                j_last = (QW // P) * (qb + 1) - 1 if causal else KC - 1
                o_t = opool.tile([DH + 1, QW], F32, tag="o", name=f"o{qb}")
                p_tiles = {}
                for t in range((j_last + 1) // 2):
                    js = (2 * t, 2 * t + 1)
                    # the pair's S matmuls are adjacent so they row-pack
                    # (rows 0-63 / 64-127 run concurrent); PVs chain after
                    for idx, j in enumerate(js):
                        emit_s(j, qb, idx, p_tiles)
                    for j in js:
                        p_sb, q_off, width = p_tiles.pop((j, qb))
                        nc.tensor.matmul(
                            o_t[:, q_off:QW],
                            v_sb[:, j, :],
                            p_sb[:, :width],
                            start=(j == 0),
                            stop=(j == j_last),
                        )

                # epilogue for this q-block
                oT = epipool.tile([DH + 1, QW], F32, tag="ot")
                nc.vector.tensor_copy(oT[:], o_t[:])
                etp = jpool.tile([P, NQB, DH + 1], F32, tag="j")
                for i in range(NQB):
                    nc.tensor.transpose(
                        etp[:, i, :], oT[:, i * P:(i + 1) * P],
                        ident[:DH + 1, :DH + 1],
                    )
                recip = epipool.tile([P, NQB], F32, tag="recip")
                nc.vector.reciprocal(recip[:], etp[:, :, DH])
                o_sb = epipool.tile([P, NQB, DH], BF16, tag="osb")
                for i in range(NQB):
                    nc.vector.tensor_scalar_mul(
                        o_sb[:, i, :], etp[:, i, :DH], recip[:, i:i + 1]
                    )
                nc.gpsimd.dma_start(
                    out_d.ap()[qb * QW:(qb + 1) * QW, :].rearrange(
                        "(i p) d -> p i d", p=P
                    ),
                    o_sb[:],
                )

    nc.compile()
    return nc


def _get(causal: bool, has_padding: bool):
    key = (causal, has_padding)
    if key not in _BUILD_CACHE:
        _BUILD_CACHE[key] = _build(causal, has_padding)
    return _BUILD_CACHE[key]


def _pack_x(x, dtype):
    # x [N, D] f32 -> x.T [D, N] with d = c*P + p, n = nh*QW + q
    # -> [p, nh, c, q], so each (p, nh) line is [c, q] contiguous (2-4 KiB)
    xt = np.asarray(x, dtype=np.float32).T.astype(dtype)        # [D, N]
    xt = xt.reshape(EC, P, NQB, QW).transpose(1, 2, 0, 3)       # [p, nh, c, q]
    return np.ascontiguousarray(xt).reshape(P, NQB * EC * QW)


def run(key_input, query_input, value_input, padding_mask, masked_attention,
        W_key, W_query=None, W_value=None, trace=False, **_ignored):
    key_input = np.asarray(key_input, dtype=np.float32)
    query_input = np.asarray(query_input, dtype=np.float32)
    value_input = np.asarray(value_input, dtype=np.float32)
    padding_mask = np.asarray(padding_mask)
    W_key = np.asarray(W_key, dtype=np.float32)

    B = key_input.shape[0]
    causal = bool(int(np.asarray(masked_attention)))
    has_padding = bool(padding_mask.any())
    nc = _get(causal, has_padding)

    f8 = ml_dtypes.float8_e4m3
    bf = ml_dtypes.bfloat16
    wcat = np.concatenate([W_key, W_key], axis=1).astype(f8)    # [D, 2*DH]
    w_b = np.ascontiguousarray(
        wcat.reshape(EC, P, 2 * DH).transpose(1, 0, 2)
    ).reshape(P, EC * 2 * DH)
    wv_b = np.ascontiguousarray(
        W_key.astype(bf).reshape(EC, P, DH).transpose(1, 0, 2)
    ).reshape(P, EC * DH)
    in_maps = []
    for b in range(B):
        m = {
            "xq_t": _pack_x(query_input[b], f8),
            "xk_t": _pack_x(key_input[b], f8),
            "xv_t": _pack_x(value_input[b], bf),
            "w": w_b,
            "wv": wv_b,
        }
        if has_padding:
            # multiplicative key mask in [KC, P] layout: 0 where padded
            km = (~padding_mask[b].reshape(N)).astype(np.float32)
            m["kmask"] = np.ascontiguousarray(km.reshape(KC, P))
        in_maps.append(m)

    if trace:
        _ensure_ntff_hook()
    res = run_bass_kernel_spmd(nc, in_maps, core_ids=list(range(B)), trace=trace)
    out = np.stack([np.asarray(res.results[b]["out"]) for b in range(B)], axis=0)
    return out.astype(np.float32), res


def kernel(**inputs) -> np.ndarray:
    out, _ = run(**inputs)
    return out


# revision 44
# speedup vs baseline: 1.5132x; 1.2434x over previous
"""Trainium2 Bass kernel for nn_AttentionHead (B=8, N=2048, D=512, d=64).

Reference semantics (faithful to the torch original):
    K = key_input   @ W_key        # note: W_key used for Q, K AND V
    Q = query_input @ W_key
    V = value_input @ W_key
    S = Q @ K^T / sqrt(512)        # scaled by INPUT dim, not head dim
    S = mask(padding), causal-mask if masked_attention
    out = softmax(S) @ V

Sharding: pure data parallelism over batch — core b computes batch element b.
No collectives. Host-side prep is layout only (transpose + fp8 cast); every
FLOP of the math runs on-device.

Device algorithm (per core):
  - Q/K inputs arrive fp8e4m3, V in bf16 (V-entry quantization passes ~1:1
    to the output; Q/K only perturb scores — combined ~0.7% l2), host-packed
    [p, quarter, c, q] so each DMA line is 2-4 KiB contiguous; one dma_start
    per (input, quarter), issue spread over sync/scalar/gpsimd from t=0
  - Q/K projections run fp8 DoubleRow (virtual 256-deep contraction, 2
    matmuls per 512-wide n-slice); V projects bf16 4-chunk; QT/KT
    [64->dup 128, 2048] with W host-duplicated [W|W]
  - V natural [128, 65] tiles via PE-transpose (bf16) with a ones column
    (softmax denominators come free as row 64 of the PV matmul)
  - streaming order: projection slice ns, then attention q-block qb=ns —
    attention for qb only needs data slices <= qb, so the PE instruction
    stream never blocks on DMA for later data; the engine stays dense,
    which also keeps the HAM clock gate at 2.4 GHz longer (each idle
    window it sees drops the PE to 1.2 GHz for 3.4+ us)
  - per k-chunk j: S.T tile [k=128, q<=512] = KT_j.T @ QT_qb (exact-causal
    widths); pair (2t, 2t+1) S matmuls adjacent so they row-pack into
    disjoint PE row groups; exp via ACT straight PSUM->SBUF bf16 with
    1/sqrt(512) folded in; diagonal blocks masked by DVE tri-multiply
  - O.T [65, q] += V'_j.T @ P.T accumulated in PSUM over the q-block's
    k-chunks, epilogue (PE-transpose, divide by sums row, bf16 DMA out on
    gpsimd) immediately after — o-psum recycles every q-block
"""

import math

import numpy as np
import ml_dtypes

import concourse.bass as bass
import concourse.tile as tile
from concourse import bacc, mybir
from concourse import masks
from concourse.bass_utils import run_bass_kernel_spmd

P = 128            # partitions / k-chunk size
N = 2048           # sequence length
D = 512            # embedding dim
DH = 64            # head dim
EC = D // P        # 4 e-chunks for the projection contraction
KC = N // P        # 16 k-chunks
QW = 512           # q block width
NQB = N // QW      # 4 q blocks / n slices
SCALE = 1.0 / math.sqrt(float(D))

BF16 = mybir.dt.bfloat16
F32 = mybir.dt.float32
FP8 = mybir.dt.float8e4
DR = mybir.MatmulPerfMode.DoubleRow

_BUILD_CACHE = {}

OPTS = {
    "sbufs": 4,            # s psum pool buffers
    "jbufs": 2,            # proj/transpose psum pool buffers
    "obufs": 2,            # o accumulator buffers (one live + one in epilogue)
}


def _ensure_ntff_hook():
    """Install the antenv.axon_hooks shim so trace=True works under axon."""
    try:
        import antenv.axon_hooks  # noqa: F401
        return
    except ImportError:
        pass
    import sys
    import types

    try:
        from trn_agent_boot.trn_boot import _ntff_profile_via_ctypes
        hook = _ntff_profile_via_ctypes("/opt/axon/libaxon_pjrt.so")
    except Exception:
        hook = None
    mod = types.ModuleType("antenv.axon_hooks")
    state = {"hook": hook}
    mod.get_axon_ntff_profile_hook = lambda: state["hook"]
    mod.set_axon_ntff_profile_hook = lambda h: state.update(hook=h)
    sys.modules["antenv.axon_hooks"] = mod
    import antenv

    antenv.axon_hooks = mod


def _build(causal: bool, has_padding: bool):
    nc = bacc.Bacc("TRN2", target_bir_lowering=False, debug=False, num_devices=8)

    # host layout [p, quarter, c, q]: line per (p, quarter) is [c, q],
    # 2 KiB fp8 for Q/K, 4 KiB bf16 for V (V-entry quantization passes ~1:1
    # to the output, fp8 there would blow the error budget).
    xq_d = nc.dram_tensor("xq_t", [P, NQB * EC * QW], FP8, kind="ExternalInput")
    xk_d = nc.dram_tensor("xk_t", [P, NQB * EC * QW], FP8, kind="ExternalInput")
    xv_d = nc.dram_tensor("xv_t", [P, NQB * EC * QW], BF16, kind="ExternalInput")
    # w host-packed [p, c, 2*DH] = [W | W] so the Q/K projections emit
    # [128, q] tiles whose partition halves are copies — S matmul pairs can
    # then row-pack into disjoint PE row groups with no cross-partition copy.
    w_d = nc.dram_tensor("w", [P, EC * 2 * DH], FP8, kind="ExternalInput")
    wv_d = nc.dram_tensor("wv", [P, EC * DH], BF16, kind="ExternalInput")
    if has_padding:
        km_d = nc.dram_tensor("kmask", [KC, P], F32, kind="ExternalInput")
    # bf16 output: quantization adds ~0.2% (quadrature, well inside the
    # 2e-2 gate) and halves the output DMA traffic + descriptor cost
    out_d = nc.dram_tensor("out", [N, DH], BF16, kind="ExternalOutput")

    with tile.TileContext(nc) as tc:
        with (
            tc.tile_pool(name="const", bufs=1) as cpool,
            tc.tile_pool(name="x", bufs=12) as xpool,
            tc.tile_pool(name="big", bufs=1) as bigpool,
            tc.tile_pool(name="p", bufs=8) as ppool,
            tc.tile_pool(name="epi", bufs=2) as epipool,
            tc.tile_pool(name="o", bufs=OPTS["obufs"], space="PSUM") as opool,
            tc.tile_pool(name="s", bufs=OPTS["sbufs"], space="PSUM") as spool,
            tc.tile_pool(name="j", bufs=OPTS["jbufs"], space="PSUM") as jpool,
        ):
            # --- weight + input DMAs issued before anything else so the
            # queues start moving within ~1us of kernel entry. w/wv on
            # gpsimd (tiny, needed by projection 0); q quarters lead on
            # sync so the projection pipeline unblocks earliest. Issue is
            # spread over three engines so descriptor generation
            # (~0.6-1us per call) runs concurrently. 2-4 KiB lines.
            # No PE warmup junk: measured on HW it neither holds the HAM
            # clock gate warm through the DMA window nor speeds the real
            # work — it only adds PE time. ---
            w_sb = cpool.tile([P, EC, 2 * DH], FP8)
            nc.gpsimd.dma_start(
                w_sb[:], w_d.ap().rearrange("p (c d) -> p c d", c=EC)
            )
            wv_sb = cpool.tile([P, EC, DH], BF16)
            nc.gpsimd.dma_start(
                wv_sb[:], wv_d.ap().rearrange("p (c d) -> p c d", c=EC)
            )
            if has_padding:
                km_sb = cpool.tile([P, KC], F32)
                nc.gpsimd.dma_start(km_sb[:], km_d.ap().transpose([1, 0]))

            # q/k quarters enter the queues first (1 MB total -> all S-side
            # data lands by ~13us); v (2 MB) follows and stays just ahead of
            # each q-block's PV consumption. k on scalar issues concurrently.
            engs = {"q": nc.sync, "k": nc.scalar, "v": nc.sync}
            dts = {"q": FP8, "k": FP8, "v": BF16}
            x_sb = {}
            for tname, xd in (("q", xq_d), ("k", xk_d), ("v", xv_d)):
                for nh in range(NQB):
                    t = xpool.tile([P, EC, QW], dts[tname], tag=f"x{tname}")
                    engs[tname].dma_start(
                        t[:],
                        xd.ap()[:, nh * EC * QW:(nh + 1) * EC * QW].rearrange(
                            "p (c q) -> p c q", c=EC
                        ),
                    )
                    x_sb[(tname, nh)] = t

            # --- ACT warmup (hide exp table load behind the DMA window) ---
            warm = cpool.tile([P, 1], F32)
            nc.vector.memset(warm[:], 0.0)
            nc.scalar.activation(warm[:], warm[:], mybir.ActivationFunctionType.Exp)

            ident = cpool.tile([P, P], F32)
            masks.make_identity(nc, ident[:])
            identb = cpool.tile([P, P], BF16)
            masks.make_identity(nc, identb[:])
            # upper-triangular (incl diag) 0/1 mask in [k, q] coords for the
            # causal diagonal blocks; multiply on DVE (gpsimd's slow semaphore
            # handling would sit in the exp->PV chain otherwise)
            tri = cpool.tile([P, P], BF16)
            masks.make_upper_triangular(nc, tri[:], val=1.0, diag=True)

            qt = bigpool.tile([P, N], BF16, tag="qt")   # rows 0-63 QT, 64-127 dup
            kt = bigpool.tile([P, N], BF16, tag="kt")
            vt = bigpool.tile([DH, N], BF16, tag="vt")
            v_sb = bigpool.tile([P, KC, DH + 1], BF16, tag="vn")
            # ones column (softmax denominator rider) set once, in SBUF —
            # a strided bf16 memset into PSUM fails the ISA 4B-cell check
            nc.vector.memset(v_sb[:, :, DH], 1.0)

            # --- streaming structure: projection slice ns, then attention
            # q-block qb == ns. Attention for qb only needs projected data
            # from slices <= qb, so the PE instruction stream never waits on
            # DMA for data later in the stream — the engine stays dense from
            # the first quarter's arrival, HAM warms early, and the whole
            # attention phase runs inside the warm-clock window. Each
            # o-accumulator is freed by its epilogue before the next q-block
            # needs one (obufs=2), which frees PSUM for a 4-deep S pipeline.

            def emit_qkproj(ns):
                # fp8 DoubleRow: virtual 256-deep contraction. Copies split
                # q->DVE / k->ACT so neither engine paces the jpool rotation
                # (ACT has no exp work yet this early).
                sl = slice(ns * QW, (ns + 1) * QW)
                for tname in ("q", "k"):
                    ps = jpool.tile([P, QW], F32, tag="j")
                    for cc in range(EC // 2):
                        nc.tensor.matmul(
                            ps[:P, :],
                            w_sb[:, 2 * cc:2 * cc + 2, :],
                            x_sb[(tname, ns)][:, 2 * cc:2 * cc + 2, :],
                            start=(cc == 0),
                            stop=(cc == EC // 2 - 1),
                            perf_mode=DR,
                        )
                    if tname == "q":
                        nc.vector.tensor_copy(qt[:, sl], ps[:])
                    else:
                        nc.scalar.activation(
                            kt[:, sl], ps[:],
                            mybir.ActivationFunctionType.Copy,
                        )

            def emit_vproj(ns):
                # bf16 4-chunk contraction (fp8 V would pass its quantization
                # error 1:1 to the output), then PE-transpose to natural tiles
                sl = slice(ns * QW, (ns + 1) * QW)
                ps = jpool.tile([P, QW], F32, tag="j")
                for c in range(EC):
                    nc.tensor.matmul(
                        ps[:DH, :],
                        wv_sb[:, c, :],
                        x_sb[("v", ns)][:, c, :],
                        start=(c == 0),
                        stop=(c == EC - 1),
                    )
                nc.vector.tensor_copy(vt[:, sl], ps[:DH, :])
                vtp = jpool.tile([P, NQB, DH + 2], BF16, tag="j")
                for i in range(NQB):
                    j = ns * NQB + i
                    nc.tensor.transpose(
                        vtp[:, i, :DH], vt[:, j * P:(j + 1) * P], identb[:DH, :DH]
                    )
                nc.vector.tensor_copy(
                    v_sb[:, ns * NQB:(ns + 1) * NQB, :DH], vtp[:, :, :DH]
                )

            def emit_s(j, qb, idx, p_tiles):
                base = DH * idx
                q_off = max(0, j * P - qb * QW) if causal else 0
                width = QW - q_off
                s_ps = spool.tile([P, QW], F32, tag="s", name=f"s{j}_{qb}")
                nc.tensor.matmul(
                    s_ps[:, :width],
                    kt[base:base + DH, j * P:(j + 1) * P],
                    qt[base:base + DH, qb * QW + q_off:(qb + 1) * QW],
                    start=True,
                    stop=True,
                )
                p_sb = ppool.tile([P, QW], BF16, tag="p", name=f"p{j}_{qb}")
                nc.scalar.activation(
                    p_sb[:, :width],
                    s_ps[:, :width],
                    mybir.ActivationFunctionType.Exp,
                    scale=SCALE,
                )
                if causal and qb == j // NQB:
                    # diagonal block at cols [0,128): keep q_loc >= k_loc
                    nc.vector.tensor_mul(p_sb[:, :P], p_sb[:, :P], tri[:])
                if has_padding:
                    nc.vector.tensor_scalar_mul(
                        p_sb[:, :width], p_sb[:, :width], km_sb[:, j:j + 1]
                    )
                p_tiles[(j, qb)] = (p_sb, q_off, width)

            # all q/k projections up front (their data lands first); each
            # v projection just before the q-block that first consumes it
            for ns in range(NQB):
                emit_qkproj(ns)
            for ns in range(NQB):
                emit_vproj(ns)

                qb = ns
                j_last = (QW // P) * (qb + 1) - 1 if causal else KC - 1
                o_t = opool.tile([DH + 1, QW], F32, tag="o", name=f"o{qb}")
                p_tiles = {}
                for t in range((j_last + 1) // 2):
                    js = (2 * t, 2 * t + 1)
                    # the pair's S matmuls are adjacent so they row-pack
                    # (rows 0-63 / 64-127 run concurrent); PVs chain after
                    for idx, j in enumerate(js):
                        emit_s(j, qb, idx, p_tiles)
                    for j in js:
                        p_sb, q_off, width = p_tiles.pop((j, qb))
                        nc.tensor.matmul(
                            o_t[:, q_off:QW],
                            v_sb[:, j, :],
                            p_sb[:, :width],
                            start=(j == 0),
                            stop=(j == j_last),
                        )

                # epilogue for this q-block
                oT = epipool.tile([DH + 1, QW], F32, tag="ot")
                nc.vector.tensor_copy(oT[:], o_t[:])
                etp = jpool.tile([P, NQB, DH + 1], F32, tag="j")
                for i in range(NQB):
                    nc.tensor.transpose(
                        etp[:, i, :], oT[:, i * P:(i + 1) * P],
                        ident[:DH + 1, :DH + 1],
                    )
                recip = epipool.tile([P, NQB], F32, tag="recip")
                nc.vector.reciprocal(recip[:], etp[:, :, DH])
                o_sb = epipool.tile([P, NQB, DH], BF16, tag="osb")
                for i in range(NQB):
                    nc.vector.tensor_scalar_mul(
                        o_sb[:, i, :], etp[:, i, :DH], recip[:, i:i + 1]
                    )
                nc.gpsimd.dma_start(
                    out_d.ap()[qb * QW:(qb + 1) * QW, :].rearrange(
                        "(i p) d -> p i d", p=P
                    ),
                    o_sb[:],
                )

    nc.compile()
    return nc


def _get(causal: bool, has_padding: bool):
    key = (causal, has_padding)
    if key not in _BUILD_CACHE:
        _BUILD_CACHE[key] = _build(causal, has_padding)
    return _BUILD_CACHE[key]


def _pack_x(x, dtype):
    # x [N, D] f32 -> x.T [D, N] with d = c*P + p, n = nh*QW + q
    # -> [p, nh, c, q], so each (p, nh) line is [c, q] contiguous (2-4 KiB)
    xt = np.asarray(x, dtype=np.float32).T.astype(dtype)        # [D, N]
    xt = xt.reshape(EC, P, NQB, QW).transpose(1, 2, 0, 3)       # [p, nh, c, q]
    return np.ascontiguousarray(xt).reshape(P, NQB * EC * QW)


def run(key_input, query_input, value_input, padding_mask, masked_attention,
        W_key, W_query=None, W_value=None, trace=False, **_ignored):
    key_input = np.asarray(key_input, dtype=np.float32)
    query_input = np.asarray(query_input, dtype=np.float32)
    value_input = np.asarray(value_input, dtype=np.float32)
    padding_mask = np.asarray(padding_mask)
    W_key = np.asarray(W_key, dtype=np.float32)

    B = key_input.shape[0]
    causal = bool(int(np.asarray(masked_attention)))
    has_padding = bool(padding_mask.any())
    nc = _get(causal, has_padding)

    f8 = ml_dtypes.float8_e4m3
    bf = ml_dtypes.bfloat16
    wcat = np.concatenate([W_key, W_key], axis=1).astype(f8)    # [D, 2*DH]
    w_b = np.ascontiguousarray(
        wcat.reshape(EC, P, 2 * DH).transpose(1, 0, 2)
    ).reshape(P, EC * 2 * DH)
    wv_b = np.ascontiguousarray(
        W_key.astype(bf).reshape(EC, P, DH).transpose(1, 0, 2)
    ).reshape(P, EC * DH)
    in_maps = []
    for b in range(B):
        m = {
            "xq_t": _pack_x(query_input[b], f8),
            "xk_t": _pack_x(key_input[b], f8),
            "xv_t": _pack_x(value_input[b], bf),
            "w": w_b,
            "wv": wv_b,
        }
        if has_padding:
            # multiplicative key mask in [KC, P] layout: 0 where padded
            km = (~padding_mask[b].reshape(N)).astype(np.float32)
            m["kmask"] = np.ascontiguousarray(km.reshape(KC, P))
        in_maps.append(m)

    if trace:
        _ensure_ntff_hook()
    res = run_bass_kernel_spmd(nc, in_maps, core_ids=list(range(B)), trace=trace)
    out = np.stack([np.asarray(res.results[b]["out"]) for b in range(B)], axis=0)
    return out.astype(np.float32), res


def kernel(**inputs) -> np.ndarray:
    out, _ = run(**inputs)
    return out
